# revision 18
# baseline (speedup 1.0000x reference)
"""Trainium2 Bass kernel for the MACE-style symmetric contraction.

Math (per node b):
    g0 = x0[i0[b]].reshape(C, U)              # element-dependent weight table
    x  = x1[b].reshape(S, U)
    for each path p (degree d in {1,2,3}):
        t = c_p * g0[w_p] * prod_k x[idx_p[k]]          # elementwise over U
        out[b, o_p] += t                                 # 16 output segments

Strategy:
  - Data-parallel over nodes on 8 cores (one SPMD NEFF, per-core data).
  - Host: sort nodes by element id so every device chunk (NC nodes) is
    element-uniform; the per-path x0 factor is then a per-partition scalar.
  - Layout on device: U=128 on partitions, nodes on the free dim.  Host
    pre-transposes x1 into [S, U, nodes] (bf16) so every DMA is dense.
  - Per path, fused DVE ops: scalar_tensor_tensor computes
    (in0 * scalar) op in1 in one instruction, folding the x0-table scale
    into the product chain.  Monomial products run in bf16 (DVE 2x mode);
    accumulation stays in fp32.
  - Every chunk gets a FRESH X tile slot and its own output DRAM tensor:
    DMA instructions have a single ISA wait slot, so slot recycling or
    same-tensor stores (2+ waits) do not compile.
"""

import numpy as np
import ml_dtypes

import concourse.bass as bass
import concourse.mybir as mybir
from concourse.tile import TileContext
from concourse import bass_utils
from concourse import tile as _tile


def _split_drain_and_barrier(self, tick_clock, wait_clock):
    """TileContext tail drain, but with the global-clock waits split across
    chained Drain instructions: this kernel touches DVE + Pool + 16 DMA
    lanes, and a single Drain cannot legally carry that many sync waits."""
    from concourse.vector_clock import ScopedClock, VectorClock

    gc = tick_clock.global_clock
    n = len(gc)
    cap = 1
    emitted = False
    i = 0
    while i < n:
        sub = [0] * n
        cnt = 0
        while i < n and cnt < cap:
            if gc[i] > 0:
                sub[i] = gc[i]
                cnt += 1
            i += 1
        if cnt:
            d = self.nc.sync.drain()
            wait_clock.add_sem_waits(d.ins, ScopedClock({None: VectorClock(sub)}))
            emitted = True
    if not emitted:
        self.nc.sync.drain()
    self.nc.all_engine_barrier()
    popped = self.nc._tile_sem_poison_stack.pop()
    assert popped is self._sem_poison
    self.nc.clear_and_free_semaphores(list(self.sems.allocated().values()))
    self.nc.all_engine_barrier()


_tile.TileContext._drain_and_barrier = _split_drain_and_barrier

U = 128
NSEG = 16
NCORES = 8
K_CHUNKS = 8      # chunks per core; 8 stores stay on distinct HWDGE lanes
F32 = mybir.dt.float32
BF16 = mybir.dt.bfloat16
MULT = mybir.AluOpType.mult
ADD = mybir.AluOpType.add


def _build_paths(idx1, w1, o1, c1, idx2, w2, o2, c2, idx3, w3, o3, c3):
    paths = []
    for idx, w, o, c in ((idx1, w1, o1, c1), (idx2, w2, o2, c2), (idx3, w3, o3, c3)):
        idx = np.asarray(idx).astype(np.int64)
        w = np.asarray(w).astype(np.int64)
        o = np.asarray(o).astype(np.int64)
        c = np.asarray(c).astype(np.float64)
        for pi in range(idx.shape[0]):
            paths.append((tuple(int(v) for v in idx[pi]), int(w[pi]), int(o[pi]),
                          float(c[pi])))
    return paths


def _build_program(paths):
    """Static op program shared by all chunks.

    Ops:
      ('pair', key, a, b)            pair tile <- X[a] * X[b]
      ('d1', o, a, p, init)          O[o] (=|+=) X[a] * T[:,p]
      ('d2', o, a, p, b, init)       O[o] (=|+=) (X[a]*T[:,p]) * X[b]
      ('d3', o, key, p, c, init)     O[o] (=|+=) (pair*T[:,p]) * X[c]
    """
    deg1 = [(i, pt) for i, pt in enumerate(paths) if len(pt[0]) == 1]
    deg2 = [(i, pt) for i, pt in enumerate(paths) if len(pt[0]) == 2]
    deg3 = [(i, pt) for i, pt in enumerate(paths) if len(pt[0]) == 3]

    # greedy pair cover for degree-3 paths
    remaining = {i: tuple(sorted(pt[0])) for i, pt in deg3}
    assign = {}
    while remaining:
        cnt = {}
        for i, f in remaining.items():
            for pr in {(f[0], f[1]), (f[0], f[2]), (f[1], f[2])}:
                cnt[pr] = cnt.get(pr, 0) + 1
        best = max(cnt, key=lambda k: cnt[k])
        for i in [i for i, f in remaining.items()
                  if best in {(f[0], f[1]), (f[0], f[2]), (f[1], f[2])}]:
            assign[i] = best
            del remaining[i]

    program = []
    seg_written = set()

    def init_flag(o):
        new = o not in seg_written
        seg_written.add(o)
        return new

    # deg3 first: each chunk then starts with a plain TensorTensor (pair)
    # op, which absorbs the X-load DMA wait on the DVE clock so later
    # TensorScalarPtr ops (1 wait slot in the ISA struct) stay legal.
    groups = {}
    for i, pt in deg3:
        groups.setdefault(assign[i], []).append((i, pt))
    for key, members in groups.items():
        program.append(('pair', key, key[0], key[1]))
        for i, (f, w, o, c) in members:
            rest = list(f)
            for v in key:
                rest.remove(v)
            program.append(('d3', o, key, i, rest[0], init_flag(o)))
    for i, (f, w, o, c) in deg2:
        program.append(('d2', o, f[0], i, f[1], init_flag(o)))
    for i, (f, w, o, c) in deg1:
        program.append(('d1', o, f[0], i, init_flag(o)))
    unused = [o for o in range(NSEG) if o not in seg_written]
    return program, unused


def _prepare(x0, i0, x1, idx1, w1, o1, c1, idx2, w2, o2, c2, idx3, w3, o3, c3):
    x0 = np.ascontiguousarray(np.asarray(x0, dtype=np.float32))
    x1 = np.ascontiguousarray(np.asarray(x1, dtype=np.float32))
    i0v = np.asarray(i0).astype(np.int64)
    B = x1.shape[0]
    NX0 = x0.shape[0]
    C = x0.shape[1] // U
    S = x1.shape[1] // U

    paths = _build_paths(idx1, w1, o1, c1, idx2, w2, o2, c2, idx3, w3, o3, c3)
    NP = len(paths)
    w_arr = np.array([p[1] for p in paths], dtype=np.int64)
    c_arr = np.array([p[3] for p in paths], dtype=np.float32)

    # per-element path-weight table with the coefficient folded in:
    # T_all[e, u, p] = c_p * x0[e, w_p, u]
    x0r = x0.reshape(NX0, C, U)
    T_all = np.ascontiguousarray(
        (x0r[:, w_arr, :] * c_arr[None, :, None]).transpose(0, 2, 1)
    ).astype(np.float32)                                   # [NX0, U, NP]

    # ---- sort nodes by element, build element-uniform chunks ----
    order = np.argsort(i0v, kind='stable')
    sorted_elems = i0v[order]
    counts = [int((i0v == e).sum()) for e in range(NX0)]
    # smallest NC (multiple of 32) fitting everything into 64 uniform chunks
    NC = 32
    while sum(-(-c // NC) for c in counts if c) > NCORES * K_CHUNKS:
        NC += 32
    chunk_elem = []
    node_ids = []
    for e in range(NX0):
        nodes_e = order[sorted_elems == e]
        for st in range(0, len(nodes_e), NC):
            blk = nodes_e[st:st + NC]
            pad = np.full(NC, -1, dtype=np.int64)
            pad[:len(blk)] = blk
            node_ids.append(pad)
            chunk_elem.append(e)
    while len(chunk_elem) < NCORES * K_CHUNKS:
        node_ids.append(np.full(NC, -1, dtype=np.int64))
        chunk_elem.append(0)
    n_chunks = len(chunk_elem)
    K = K_CHUNKS
    KNC = K * NC
    node_ids = np.concatenate(node_ids)
    chunk_elem = np.array(chunk_elem, dtype=np.int64)

    valid = node_ids >= 0
    x1_sorted = np.zeros((n_chunks * NC, S * U), dtype=np.float32)
    x1_sorted[valid] = x1[node_ids[valid]]

    in_maps = []
    for r in range(NCORES):
        blk = x1_sorted[r * KNC:(r + 1) * KNC]             # [KNC, S*U]
        x1t = np.ascontiguousarray(
            blk.T.reshape(S, U, KNC)).astype(ml_dtypes.bfloat16)
        ttab = np.ascontiguousarray(T_all[chunk_elem[r * K:(r + 1) * K]])
        in_maps.append({'x1t': x1t, 'ttab': ttab})

    program, unused_segs = _build_program(paths)

    # ---- build the bass kernel ----
    nc = bass.Bass("TRN2")
    x1t_d = nc.dram_tensor("x1t", [S, U, KNC], BF16, kind="ExternalInput")
    ttab_d = nc.dram_tensor("ttab", [K, U, NP], F32, kind="ExternalInput")
    out_ds = [nc.dram_tensor(f"out_t{ck}", [NSEG, U, NC], F32,
                             kind="ExternalOutput") for ck in range(K)]

    with TileContext(nc) as tc:
        with tc.tile_pool(name="xp", bufs=2) as xp, \
             tc.tile_pool(name="op", bufs=2) as op_, \
             tc.tile_pool(name="tp", bufs=4) as tp:
            Tt_all = xp.tile([U, K * NP], F32, tag="ttab", bufs=1,
                             name="ttab_all")
            nc.gpsimd.dma_start(
                Tt_all[:].rearrange("u (k p) -> u k p", p=NP),
                ttab_d.rearrange("k u p -> u k p"))
            # dummy TensorTensor absorbs the ttab DMA wait on the DVE clock:
            # TensorScalarPtr ops have a single ISA wait slot and must not
            # inherit this DMA dependency directly.
            warm = xp.tile([U, 1], F32, tag="warm", bufs=1, name="warm0")
            nc.vector.tensor_max(warm[:], Tt_all[:, 0:1], Tt_all[:, 0:1])
            for ck in range(K):
                c0, c1 = ck * NC, (ck + 1) * NC
                # fresh slot per chunk: a recycled slot would put 2+ waits on
                # the 1-wait-slot DMA instruction
                X_all = xp.tile([U, S * NC], BF16, tag=f"xall{ck}", bufs=1,
                                name=f"xall_{ck}")
                nc.gpsimd.dma_start(
                    X_all[:], x1t_d[:, :, c0:c1].rearrange("s u n -> u s n"))
                X = [X_all[:, s * NC:(s + 1) * NC] for s in range(S)]
                Tt = Tt_all[:, ck * NP:(ck + 1) * NP]

                O_all = op_.tile([U, NSEG * NC], F32, tag="oall",
                                 name=f"oall_{ck}")
                O = {o: O_all[:, o * NC:(o + 1) * NC] for o in range(NSEG)}
                pair_tiles = {}
                for ni, op in enumerate(program):
                    kind = op[0]
                    if kind == 'pair':
                        _, key, a, b = op
                        t = tp.tile([U, NC], BF16, tag="pair",
                                    name=f"pr_{ck}_{ni}")
                        nc.vector.tensor_mul(t[:], X[a], X[b])
                        pair_tiles[key] = t
                    elif kind == 'd1':
                        _, o, a, p, init = op
                        col = Tt[:, p:p + 1]
                        if init:
                            nc.vector.tensor_scalar_mul(O[o], X[a], col)
                        else:
                            nc.vector.scalar_tensor_tensor(
                                O[o], X[a], col, O[o], MULT, ADD)
                    elif kind == 'd2':
                        _, o, a, p, b, init = op
                        col = Tt[:, p:p + 1]
                        if init:
                            nc.vector.scalar_tensor_tensor(
                                O[o], X[a], col, X[b], MULT, MULT)
                        else:
                            q = tp.tile([U, NC], BF16, tag="q",
                                        name=f"q_{ck}_{ni}")
                            nc.vector.scalar_tensor_tensor(
                                q[:], X[a], col, X[b], MULT, MULT)
                            nc.vector.tensor_add(O[o], O[o], q[:])
                    elif kind == 'd3':
                        _, o, key, p, cidx, init = op
                        col = Tt[:, p:p + 1]
                        pt = pair_tiles[key]
                        if init:
                            nc.vector.scalar_tensor_tensor(
                                O[o], pt[:], col, X[cidx], MULT, MULT)
                        else:
                            q = tp.tile([U, NC], BF16, tag="q",
                                        name=f"q_{ck}_{ni}")
                            nc.vector.scalar_tensor_tensor(
                                q[:], pt[:], col, X[cidx], MULT, MULT)
                            nc.vector.tensor_add(O[o], O[o], q[:])
                for o in unused_segs:
                    nc.vector.memset(O[o], 0.0)
                nc.sync.dma_start(
                    out_ds[ck].rearrange("o u n -> u o n"), O_all[:])

    meta = dict(B=B, K=K, NC=NC, node_ids=node_ids, valid=valid)
    return nc, in_maps, meta


def _assemble(results, meta):
    B, K, NC = meta['B'], meta['K'], meta['NC']
    node_ids, valid = meta['node_ids'], meta['valid']
    out_sorted = np.concatenate(
        [np.concatenate(
            [results[r][f'out_t{ck}'].transpose(2, 0, 1).reshape(NC, NSEG * U)
             for ck in range(K)], axis=0)
         for r in range(NCORES)], axis=0)
    out_full = np.zeros((B, NSEG * U), dtype=np.float32)
    out_full[node_ids[valid]] = out_sorted[valid]
    return out_full


def kernel(**inputs):
    nc, in_maps, meta = _prepare(**inputs)
    res = bass_utils.run_bass_kernel_spmd(
        nc, in_maps, core_ids=list(range(NCORES)), trace=False)
    return _assemble(res.results, meta)
